# revision 20
# baseline (speedup 1.0000x reference)
"""Trainium2 Bass kernel for EnhancedCondConv2d (moe_routing).

Data-parallel over batch: 8 cores x 2 samples each. Full inputs in,
full outputs back.

Per-core program (per sample):
  1. routing: avgpool(x) -> tiny MLP -> softmax -> rweights [16]
  2. w[b] = sum_e rweights[e] * experts[e]  (block-diag PE matmuls)
  3. 3x3 grouped conv as 9 PSUM-accumulated shifted matmuls (bf16).
     x is stored UNPADDED (contiguous rows, line-rate DMA) with only
     guard columns; the W-edge wrap contaminates output columns 0 and
     127, which are recomputed exactly via 12 small edge matmuls and
     evicted separately (so SE channel sums stay exact).
  4. SE: channel mean folded into PSUM eviction (ACT accum), MLP -> cw,
     in-place bf16 DVE scale pass osb *= cw
  5. CBAM: PE transposes (bf16) -> DVE max / sum over channels -> 7x7
     conv as 14 banded-Toeplitz matmuls (host-precomputed bands)
  6. final: out = osb*sw + x in bf16 (contiguous reads), ACT cast to
     f32, DMA out

Pipelining: sample 0's CBAM/final chunks are emitted interleaved with
sample 1's conv supers so sample 0's DVE-heavy post-conv phase hides
under sample 1's PE-heavy conv.
"""

import math
from contextlib import ExitStack

import numpy as np

import concourse.bass as bass
import concourse.bacc as bacc
import concourse.mybir as mybir
import concourse.tile as tile
from concourse.bass_utils import run_bass_kernel_spmd

F32 = mybir.dt.float32
BF16 = mybir.dt.bfloat16
AX = mybir.AxisListType
ALU = mybir.AluOpType
ACTF = mybir.ActivationFunctionType

B, CI, CO, H, W, E, KK, RR = 16, 128, 128, 128, 128, 16, 3, 8
NCORES = 8
BL = B // NCORES  # 2 samples per core
EPS = 1e-5
HW = H * W
IKK = CI * KK * KK  # 1152
BNS = 1.0 / math.sqrt(1.0 + EPS)
# unpadded x layout: [2 guard][row0 zeros][rows 1..128 = x][row129 zeros][2 guard]
GF = 2                      # front guard elems (keeps rows 4B-aligned)
XROW = lambda r: GF + r * W  # flat offset of padded row r (0..129)
# extra tail slack so strided edge-column APs (base + h*W, h<128) stay
# in-bounds; the slack region itself is never read
GL = GF + 131 * W

_CACHE = {}


def _build_module():
    nc = bacc.Bacc("TRN2", target_bir_lowering=False, debug=False)

    # ---- external inputs (host-prepped layouts) ----
    xb_d = nc.dram_tensor("x2b", [BL, CI, H, W], BF16, kind="ExternalInput").ap()
    ew_d = nc.dram_tensor("experts_w", [16, 128, IKK], BF16, kind="ExternalInput").ap()
    ident_d = nc.dram_tensor("identb", [128, 128], BF16, kind="ExternalInput").ap()
    mcdh_d = nc.dram_tensor("mcdh", [128, 14 * 128], BF16, kind="ExternalInput").ap()
    emask_d = nc.dram_tensor("emask", [128, 16], F32, kind="ExternalInput").ap()
    rw1t_d = nc.dram_tensor("rw1t", [CI, 16], F32, kind="ExternalInput").ap()
    rw2t_d = nc.dram_tensor("rw2t", [16, CI], F32, kind="ExternalInput").ap()
    rw3t_d = nc.dram_tensor("rw3t", [CI, 16], F32, kind="ExternalInput").ap()
    caw1t_d = nc.dram_tensor("caw1t", [CO, 16], F32, kind="ExternalInput").ap()
    caw2t_d = nc.dram_tensor("caw2t", [16, CO], F32, kind="ExternalInput").ap()
    g1_d = nc.dram_tensor("rbn1_g", [16], F32, kind="ExternalInput").ap()
    b1_d = nc.dram_tensor("rbn1_b", [16], F32, kind="ExternalInput").ap()
    g2_d = nc.dram_tensor("rbn2_g", [CI], F32, kind="ExternalInput").ap()
    b2_d = nc.dram_tensor("rbn2_b", [CI], F32, kind="ExternalInput").ap()
    rb3_d = nc.dram_tensor("rb3", [E], F32, kind="ExternalInput").ap()
    cag1_d = nc.dram_tensor("ca_bn1_g", [16], F32, kind="ExternalInput").ap()
    cab1_d = nc.dram_tensor("ca_bn1_b", [16], F32, kind="ExternalInput").ap()
    cag2_d = nc.dram_tensor("ca_bn2_g", [CO], F32, kind="ExternalInput").ap()
    cab2_d = nc.dram_tensor("ca_bn2_b", [CO], F32, kind="ExternalInput").ap()
    sag_d = nc.dram_tensor("sa_bn_g", [1], F32, kind="ExternalInput").ap()
    sab_d = nc.dram_tensor("sa_bn_b", [1], F32, kind="ExternalInput").ap()
    bmask_d = nc.dram_tensor("bmask", [128, 8], BF16, kind="ExternalInput").ap()

    out_d = nc.dram_tensor("out", [BL, CO, H, W], F32, kind="ExternalOutput").ap()

    # internal DRAM scratch (spatial-attention map bounce for broadcast)
    ssw_d = nc.dram_tensor("scr_sw", [BL, H, W], BF16).ap()

    with tile.TileContext(nc) as tc, ExitStack() as ctx:
        _kernel_body(
            ctx, tc,
            xb_d, ew_d, ident_d, mcdh_d, emask_d, rw1t_d, rw2t_d, rw3t_d,
            caw1t_d, caw2t_d, g1_d, b1_d, g2_d, b2_d, rb3_d, cag1_d, cab1_d,
            cag2_d, cab2_d, sag_d, sab_d, bmask_d, out_d, ssw_d,
        )
    nc.compile()
    return nc


def _kernel_body(ctx, tc,
                 xb_d, ew_d, ident_d, mcdh_d, emask_d, rw1t_d, rw2t_d, rw3t_d,
                 caw1t_d, caw2t_d, g1_d, b1_d, g2_d, b2_d, rb3_d, cag1_d,
                 cab1_d, cag2_d, cab2_d, sag_d, sab_d, bmask_d, out_d, ssw_d):
    nc = tc.nc

    cpool = ctx.enter_context(tc.tile_pool(name="const", bufs=1))
    xpool = ctx.enter_context(tc.tile_pool(name="xp", bufs=2))
    opool = ctx.enter_context(tc.tile_pool(name="op", bufs=2))
    wpool = ctx.enter_context(tc.tile_pool(name="wp", bufs=2))
    epool = ctx.enter_context(tc.tile_pool(name="ep", bufs=12))
    spool = ctx.enter_context(tc.tile_pool(name="sp", bufs=2))
    fpre = ctx.enter_context(tc.tile_pool(name="fpre", bufs=2))
    fpool = ctx.enter_context(tc.tile_pool(name="fp", bufs=2))
    fof = ctx.enter_context(tc.tile_pool(name="fof", bufs=2))

    pconv = ctx.enter_context(tc.tile_pool(name="pc", bufs=4, space="PSUM"))
    ptp = ctx.enter_context(tc.tile_pool(name="pt", bufs=2, space="PSUM"))
    pw = ctx.enter_context(tc.tile_pool(name="pw", bufs=1, space="PSUM"))
    pr = ctx.enter_context(tc.tile_pool(name="prt", bufs=1, space="PSUM"))

    # ---------- x load for sample 0 first: its DMAs + pool partial sums
    # must not queue behind the constant-loading storm ----------
    xus = [None, None]
    pparts_t = [None, None]

    def stage_load_pool(b):
        xu = xpool.tile([128, GL], BF16, tag="x_un", name=f"xu{b}")
        xus[b] = xu
        nc.vector.memset(xu[:, 0:XROW(1)], 0.0)
        nc.vector.memset(xu[:, XROW(129):GL], 0.0)
        pparts = spool.tile([128, 8], F32, tag="pparts", name=f"pparts{b}")
        pparts_t[b] = pparts
        for t in range(8):
            lo = XROW(1 + 16 * t)
            chunk = xu[:, lo:lo + 16 * W]
            nc.sync.dma_start(chunk, xb_d[b, :, 16 * t:16 * t + 16, :])
            if t % 2 == 0:
                nc.vector.tensor_reduce(pparts[:, t:t + 1], chunk, AX.X,
                                        ALU.add)
            else:
                # in-place ACT copy; accum_out collects the chunk sum
                nc.scalar.activation(chunk, chunk, ACTF.Copy,
                                     accum_out=pparts[:, t:t + 1])

    stage_load_pool(0)

    # ---------- constants ----------
    ident = cpool.tile([128, 128], BF16, tag="ident")
    nc.sync.dma_start(ident, ident_d)
    mcdh = cpool.tile([128, 14 * 128], BF16, tag="mcdh")
    nc.sync.dma_start(mcdh, mcdh_d)
    emask = cpool.tile([128, 16], F32, tag="emask")
    nc.sync.dma_start(emask, emask_d)
    ones1 = cpool.tile([1, 128], F32, tag="ones1")
    nc.vector.memset(ones1, 1.0)

    rw1t = cpool.tile([CI, 16], F32, tag="rw1t")
    nc.sync.dma_start(rw1t, rw1t_d)
    rw2t = cpool.tile([16, CI], F32, tag="rw2t")
    nc.sync.dma_start(rw2t, rw2t_d)
    rw3t = cpool.tile([CI, 16], F32, tag="rw3t")
    nc.sync.dma_start(rw3t, rw3t_d)
    caw1t = cpool.tile([CO, 16], F32, tag="caw1t")
    nc.sync.dma_start(caw1t, caw1t_d)
    caw2t = cpool.tile([16, CO], F32, tag="caw2t")
    nc.sync.dma_start(caw2t, caw2t_d)

    def vec_const(dst_tag, src_ap, n, scale):
        raw = cpool.tile([n, 1], F32, tag=dst_tag + "_r")
        nc.sync.dma_start(raw, src_ap.unsqueeze(1))
        out = cpool.tile([n, 1], F32, tag=dst_tag)
        nc.vector.tensor_scalar_mul(out, raw, float(scale))
        return out

    gs1 = vec_const("gs1", g1_d, 16, BNS / HW)
    bb1 = vec_const("bb1", b1_d, 16, 1.0)
    gs2 = vec_const("gs2", g2_d, CI, BNS)
    bb2 = vec_const("bb2", b2_d, CI, 1.0)
    gsca1 = vec_const("gsca1", cag1_d, 16, BNS / HW)
    bbca1 = vec_const("bbca1", cab1_d, 16, 1.0)
    gsca2 = vec_const("gsca2", cag2_d, CO, BNS)
    bbca2 = vec_const("bbca2", cab2_d, CO, 1.0)

    rb3r = cpool.tile([1, E], F32, tag="rb3r")
    nc.sync.dma_start(rb3r, rb3_d.unsqueeze(0))

    gssa = cpool.tile([128, 1], F32, tag="gssa")
    nc.sync.dma_start(gssa, sag_d.unsqueeze(0).partition_broadcast(128))
    nc.vector.tensor_scalar_mul(gssa, gssa, BNS)
    bssa = cpool.tile([128, 1], F32, tag="bssa")
    nc.sync.dma_start(bssa, sab_d.unsqueeze(0).partition_broadcast(128))
    bmask = cpool.tile([128, 8], BF16, tag="bmask")
    nc.sync.dma_start(bmask, bmask_d)

    # ---------- per-sample state ----------
    osbs = [None, None]
    wsbs = [None, None]
    cparts = [None, None]
    stats = [None, None]

    def stage_route(b):
        """routing MLP -> softmax -> block-diag rwblk."""
        pparts = pparts_t[b]
        psum_t = spool.tile([128, 1], F32, tag="psum_t")
        nc.scalar.activation(pparts, pparts, ACTF.Copy, accum_out=psum_t)

        # -- routing MLP (f32) --
        mm1 = pr.tile([16, 1], F32, tag="r")
        nc.tensor.matmul(mm1, rw1t, psum_t, start=True, stop=True)
        h1 = spool.tile([16, 1], F32, tag="h1")
        nc.scalar.activation(h1, mm1, ACTF.Relu, bias=bb1, scale=gs1)
        mm2 = pr.tile([128, 1], F32, tag="r")
        nc.tensor.matmul(mm2, rw2t, h1, start=True, stop=True)
        gg = spool.tile([128, 1], F32, tag="gg")
        nc.scalar.activation(gg, mm2, ACTF.Sigmoid, bias=bb2, scale=gs2)
        mm3 = pr.tile([1, E], F32, tag="r")
        nc.tensor.matmul(mm3, gg, rw3t, start=True, stop=True)
        lg = spool.tile([1, E], F32, tag="lg")
        nc.vector.tensor_add(lg, mm3, rb3r)
        mx = spool.tile([1, 1], F32, tag="mx")
        nc.vector.tensor_reduce(mx, lg, AX.X, ALU.max)
        mxn = spool.tile([1, 1], F32, tag="mxn")
        nc.vector.tensor_scalar_mul(mxn, mx, -1.0)
        e16 = spool.tile([1, E], F32, tag="e16")
        nc.scalar.activation(e16, lg, ACTF.Exp, bias=mxn, scale=1.0)
        s1 = spool.tile([1, 1], F32, tag="s1")
        nc.vector.tensor_reduce(s1, e16, AX.X, ALU.add)
        rinv = spool.tile([1, 1], F32, tag="rinv")
        nc.vector.reciprocal(rinv, s1)
        rwrow = spool.tile([1, E], F32, tag="rwrow")
        nc.vector.tensor_scalar_mul(rwrow, e16, rinv)

        # on-chip broadcast: rank-1 PE outer product + masked select
        rwbp = pr.tile([128, 16], F32, tag="r")
        nc.tensor.matmul(rwbp, ones1, rwrow, start=True, stop=True)
        rwsel = spool.tile([128, 16], F32, tag="rwsel")
        nc.vector.tensor_mul(rwsel, rwbp, emask)
        rwcol = spool.tile([128, 1], F32, tag="rwcol")
        nc.vector.tensor_reduce(rwcol, rwsel, AX.X, ALU.add)
        rwblk = spool.tile([128, 8], BF16, tag="rwblk")
        nc.vector.tensor_scalar_mul(rwblk, bmask, rwcol)
        return rwblk

    def stage_wgen(b, rwblk):
        """w[i, k, o] = sum_e rw[e] experts[e, o, i, k] via block-diag MMs."""
        wsb = wpool.tile([128, KK * KK, CO], BF16, tag="wsb")
        wsbs[b] = wsb
        for og in range(16):
            ec = epool.tile([128, IKK], BF16, tag="echunk")
            nc.sync.dma_start(ec, ew_d[og])
            eck = ec.rearrange("p (k i) -> p k i", k=9)
            pwt = pw.tile([128, 9, 8], F32, tag="w")
            for k in range(9):
                nc.tensor.matmul(pwt[:, k, :], eck[:, k, :], rwblk,
                                 start=True, stop=True)
            nc.scalar.copy(wsb[:, :, og * 8:og * 8 + 8], pwt)

    def stage_conv_alloc(b):
        osbs[b] = opool.tile([128, H, W], BF16, tag="out_sb",
                             name=f"osb{b}")
        cparts[b] = spool.tile([128, 34], F32, tag="cparts",
                               name=f"cparts{b}")

    def emit_conv_edges(b):
        """exact conv output columns 0 and 127 (the bulk matmuls read
        wrapped garbage there); evicted into osb + edge channel sums."""
        xu, wsb, osb = xus[b], wsbs[b], osbs[b]
        cp = cparts[b]
        for side, kws, wcol in ((0, (1, 2), 0), (1, (0, 1), W - 1)):
            pe = ptp.tile([128, 128], F32, tag="t", name=f"pe{b}_{side}")
            first = True
            for kh in range(3):
                for kw in kws:
                    xcol = wcol + kw - 1
                    base = XROW(kh) + xcol
                    rhs = xu[:, base:base + HW].rearrange(
                        "p (a c) -> p a c", a=128)[:, :, 0]
                    nc.tensor.matmul(pe, wsb[:, kh * 3 + kw, :], rhs,
                                     start=first, stop=(kh == 2 and kw == kws[-1]))
                    first = False
            nc.scalar.activation(osb[:, :, wcol], pe, ACTF.Copy,
                                 accum_out=cp[:, 32 + side:33 + side])

    def emit_conv_super(b, sup):
        """one 16-row super: 4 PSUM groups x 9 taps, evict interior."""
        xu, wsb, osb, cp = xus[b], wsbs[b], osbs[b], cparts[b]
        pcs = [pconv.tile([128, 512], F32, tag="c", name=f"pc{b}_{sup}_{i}")
               for i in range(4)]
        for k in range(9):
            kh, kw = k // 3, k % 3
            lhs = wsb[:, k, :]
            for g in range(4):
                base = XROW(sup * 16 + g * 4 + kh) + kw - 1
                rhs = xu[:, base:base + 4 * W].rearrange(
                    "p (a c) -> p a c", a=4)
                nc.tensor.matmul(pcs[g], lhs, rhs,
                                 start=(k == 0), stop=(k == 8))
        for g in range(4):
            hr = sup * 16 + g * 4
            src = pcs[g].rearrange("p (a c) -> p a c", a=4)[:, :, 1:W - 1]
            nc.scalar.activation(
                osb[:, hr:hr + 4, 1:W - 1], src,
                ACTF.Copy, accum_out=cp[:, sup * 4 + g:sup * 4 + g + 1])

    def stage_se(b, split_scale=False):
        """SE MLP on accumulated channel sums -> cw, in-place bf16 scale."""
        osb, cp = osbs[b], cparts[b]
        cps = spool.tile([128, 1], F32, tag="cps")
        nc.scalar.activation(cp, cp, ACTF.Copy, accum_out=cps)
        se1 = pr.tile([16, 1], F32, tag="r")
        nc.tensor.matmul(se1, caw1t, cps, start=True, stop=True)
        ch = spool.tile([16, 1], F32, tag="ch")
        nc.scalar.activation(ch, se1, ACTF.Relu, bias=bbca1, scale=gsca1)
        se2 = pr.tile([128, 1], F32, tag="r")
        nc.tensor.matmul(se2, caw2t, ch, start=True, stop=True)
        cw = spool.tile([128, 1], F32, tag="cw")
        nc.scalar.activation(cw, se2, ACTF.Sigmoid, bias=bbca2, scale=gsca2)
        for g in range(8):
            sl = osb[:, 16 * g:16 * g + 16, :]
            if split_scale and g < 2:
                nc.scalar.mul(sl, sl, cw)
            else:
                nc.vector.tensor_scalar_mul(sl, sl, cw)

    def emit_stat_init(b):
        spmax = spool.tile([128, 134], BF16, tag="spmax", name=f"spmax{b}")
        spsum = spool.tile([128, 134], BF16, tag="spsum", name=f"spsum{b}")
        stats[b] = (spmax, spsum)
        nc.vector.memset(spmax[:, 0:3], 0.0)
        nc.vector.memset(spmax[:, 131:134], 0.0)
        nc.vector.memset(spsum[:, 0:3], 0.0)
        nc.vector.memset(spsum[:, 131:134], 0.0)

    def emit_stat_chunk(b, q):
        """transpose 8 h-rows, reduce channel max / sum into [w, h] maps."""
        osb = osbs[b]
        spmax, spsum = stats[b]
        ptt = ptp.tile([128, 1024], BF16, tag="t")
        for j in range(8):
            nc.tensor.transpose(
                ptt[:, 128 * j:128 * (j + 1)], osb[:, 8 * q + j, :], ident)
        v = ptt.rearrange("p (a c) -> p a c", a=8)
        nc.vector.tensor_reduce(spmax[:, 3 + 8 * q:11 + 8 * q], v, AX.X, ALU.max)
        with nc.allow_low_precision(reason="bf16 channel-sum for 7x7 attn"):
            nc.vector.tensor_reduce(spsum[:, 3 + 8 * q:11 + 8 * q], v, AX.X,
                                    ALU.add)

    def emit_banded(b, half=None):
        """7x7 spatial conv as 14 banded-Toeplitz matmuls + sigmoid.
        half=0/1 computes only output rows [0,64) / [64,128) so the
        sw-map DRAM bounce can start before all stat chunks finish."""
        spmax, spsum = stats[b]
        lo, n = (0, 128) if half is None else (64 * half, 64)
        psw = ptp.tile([128, 128], F32, tag="t", name=f"psw{b}_{half}")
        for t in range(14):
            c, dh = t // 7, t % 7
            src = spsum if c == 0 else spmax
            nc.tensor.matmul(psw[:, 0:n], mcdh[:, t * 128:(t + 1) * 128],
                             src[:, lo + dh:lo + dh + n],
                             start=(t == 0), stop=(t == 13))
        swT = spool.tile([128, 128], BF16, tag="swT", name=f"swT{b}_{half}")
        nc.scalar.activation(swT[:, 0:n], psw[:, 0:n], ACTF.Sigmoid,
                             bias=bssa, scale=gssa)
        pswh = ptp.tile([128, 128], BF16, tag="t", name=f"pswh{b}_{half}")
        nc.tensor.transpose(pswh[0:n, :], swT[:, 0:n], ident)
        swH = spool.tile([128, 128], BF16, tag="swH", name=f"swH{b}_{half}")
        nc.vector.tensor_copy(swH[0:n, :], pswh[0:n, :])
        nc.sync.dma_start(ssw_d[b, lo:lo + n, :], swH[0:n, :])

    def emit_final_chunk(b, t):
        """out[:, 16t:16t+16, :] = osb*sw + x (bf16), cast f32, store."""
        xu, osb = xus[b], osbs[b]
        r0 = 16 * t
        eng = nc.vector
        swbc = fpre.tile([128, 16, W], BF16, tag="swbc")
        nc.sync.dma_start(
            swbc, ssw_d[b, r0:r0 + 16, :].partition_broadcast(128))
        tmul = fpool.tile([128, 16, W], BF16, tag="tmul")
        eng.tensor_tensor(tmul, osb[:, r0:r0 + 16, :], swbc, ALU.mult)
        lo = XROW(1 + r0)
        xres = xu[:, lo:lo + 16 * W].rearrange("p (a c) -> p a c", a=16)
        eng.tensor_tensor(tmul, tmul, xres, ALU.add)
        fo = fof.tile([128, 16, W], F32, tag="fo")
        nc.scalar.copy(fo, tmul)
        nc.sync.dma_start(out_d[b, :, r0:r0 + 16, :], fo)

    # ---------- pipelined schedule over the two samples ----------
    rwblk0 = stage_route(0)
    stage_wgen(0, rwblk0)
    stage_conv_alloc(0)
    emit_conv_edges(0)
    for sup in range(8):
        emit_conv_super(0, sup)
    stage_load_pool(1)
    rwblk1 = stage_route(1)
    stage_wgen(1, rwblk1)
    stage_conv_alloc(1)
    emit_conv_edges(1)
    stage_se(0)
    emit_stat_init(0)

    # sample 1 conv interleaved with sample 0 CBAM + final
    for sup in range(8):
        emit_conv_super(1, sup)
        if sup < 4:
            for qq in range(4):
                emit_stat_chunk(0, 4 * sup + qq)
        elif sup == 4:
            emit_banded(0)
        elif sup == 5:
            for t in range(4):
                emit_final_chunk(0, t)
        elif sup == 6:
            for t in range(4, 8):
                emit_final_chunk(0, t)

    # sample 1 post-conv tail
    stage_se(1, split_scale=True)
    emit_stat_init(1)
    for q in range(9):
        emit_stat_chunk(1, q)
    emit_banded(1, half=0)
    for q in range(9, 16):
        emit_stat_chunk(1, q)
    emit_banded(1, half=1)
    for t in range(8):
        emit_final_chunk(1, t)


def _host_prep(inp):
    import ml_dtypes
    experts = np.ascontiguousarray(inp["experts"], dtype=np.float32)
    # ew2[og][j'*16+e][k*128+i] = experts[e, og*8+j', i, kh, kw], k=kh*3+kw
    ew = experts.reshape(E, 16, 8, CI, 9)
    ew = np.ascontiguousarray(ew.transpose(1, 2, 0, 4, 3)).reshape(16, 128, IKK)

    identb = np.eye(128, dtype=np.float32)

    # banded-Toeplitz 7x7 attention matrices:
    # mcdh[c*7+dh][k, w] = sak[c, dh, k-w+3] for k-w+3 in [0,7), else 0
    saw = np.asarray(inp["sa_w"], np.float32).reshape(2, 7, 7)
    sak = saw.copy()
    sak[0] *= 1.0 / CO  # fold channel-mean normalization into mean taps
    mc = np.zeros((14, 128, 128), dtype=np.float32)
    kk, ww = np.meshgrid(np.arange(128), np.arange(128), indexing="ij")
    dwi = kk - ww + 3
    band = (dwi >= 0) & (dwi < 7)
    for c in range(2):
        for dh in range(7):
            m = np.zeros((128, 128), dtype=np.float32)
            m[band] = sak[c, dh, dwi[band]]
            mc[c * 7 + dh] = m
    mcdh = np.ascontiguousarray(mc.transpose(1, 0, 2)).reshape(128, 14 * 128)

    bm = np.zeros((8, 16, 8), dtype=np.float32)
    for j in range(8):
        bm[j, :, j] = 1.0
    bm = bm.reshape(128, 8)

    # emask[p, e] = 1 if e == p % 16 (block-diag expert select)
    em = np.zeros((128, 16), dtype=np.float32)
    em[np.arange(128), np.arange(128) % 16] = 1.0

    shared = {
        "experts_w": ew.astype(ml_dtypes.bfloat16),
        "identb": identb.astype(ml_dtypes.bfloat16),
        "mcdh": mcdh.astype(ml_dtypes.bfloat16),
        "emask": em,
        "rw1t": np.ascontiguousarray(inp["rw1"].T, dtype=np.float32),
        "rw2t": np.ascontiguousarray(inp["rw2"].T, dtype=np.float32),
        "rw3t": np.ascontiguousarray(inp["rw3"].T, dtype=np.float32),
        "caw1t": np.ascontiguousarray(inp["ca_w1"].T, dtype=np.float32),
        "caw2t": np.ascontiguousarray(inp["ca_w2"].T, dtype=np.float32),
        "rbn1_g": np.asarray(inp["rbn1_g"], np.float32),
        "rbn1_b": np.asarray(inp["rbn1_b"], np.float32),
        "rbn2_g": np.asarray(inp["rbn2_g"], np.float32),
        "rbn2_b": np.asarray(inp["rbn2_b"], np.float32),
        "rb3": np.asarray(inp["rb3"], np.float32),
        "ca_bn1_g": np.asarray(inp["ca_bn1_g"], np.float32),
        "ca_bn1_b": np.asarray(inp["ca_bn1_b"], np.float32),
        "ca_bn2_g": np.asarray(inp["ca_bn2_g"], np.float32),
        "ca_bn2_b": np.asarray(inp["ca_bn2_b"], np.float32),
        "sa_bn_g": np.asarray(inp["sa_bn_g"], np.float32),
        "sa_bn_b": np.asarray(inp["sa_bn_b"], np.float32),
        "bmask": bm.astype(ml_dtypes.bfloat16),
    }
    x = np.asarray(inp["x"], np.float32)
    in_maps = []
    for c in range(NCORES):
        m = dict(shared)
        xc = np.ascontiguousarray(x[BL * c:BL * (c + 1)])
        m["x2b"] = xc.astype(ml_dtypes.bfloat16)
        in_maps.append(m)
    return in_maps


def get_module():
    if "nc" not in _CACHE:
        _CACHE["nc"] = _build_module()
    return _CACHE["nc"]


def kernel(**inputs):
    nc = get_module()
    in_maps = _host_prep(inputs)
    res = run_bass_kernel_spmd(nc, in_maps, core_ids=list(range(NCORES)))
    out = np.concatenate([r["out"] for r in res.results], axis=0)
    return out.astype(np.float32)


# revision 21
# speedup vs baseline: 1.0027x; 1.0027x over previous
"""Trainium2 Bass kernel for EnhancedCondConv2d (moe_routing).

Data-parallel over batch: 8 cores x 2 samples each. Full inputs in,
full outputs back.

Per-core program (per sample):
  1. routing: avgpool(x) -> tiny MLP -> softmax -> rweights [16]
  2. w[b] = sum_e rweights[e] * experts[e]  (block-diag PE matmuls)
  3. 3x3 grouped conv as 9 PSUM-accumulated shifted matmuls (bf16).
     x is stored UNPADDED (contiguous rows, line-rate DMA) with only
     guard columns; the W-edge wrap contaminates output columns 0 and
     127, which are recomputed exactly via 12 small edge matmuls and
     evicted separately (so SE channel sums stay exact).
  4. SE: channel mean folded into PSUM eviction (ACT accum), MLP -> cw,
     in-place bf16 DVE scale pass osb *= cw
  5. CBAM: PE transposes (bf16) -> DVE max / sum over channels -> 7x7
     conv as 14 banded-Toeplitz matmuls (host-precomputed bands)
  6. final: out = osb*sw + x in bf16 (contiguous reads), ACT cast to
     f32, DMA out

Pipelining: sample 0's CBAM/final chunks are emitted interleaved with
sample 1's conv supers so sample 0's DVE-heavy post-conv phase hides
under sample 1's PE-heavy conv.
"""

import math
from contextlib import ExitStack

import numpy as np

import concourse.bass as bass
import concourse.bacc as bacc
import concourse.mybir as mybir
import concourse.tile as tile
from concourse.bass_utils import run_bass_kernel_spmd

F32 = mybir.dt.float32
BF16 = mybir.dt.bfloat16
AX = mybir.AxisListType
ALU = mybir.AluOpType
ACTF = mybir.ActivationFunctionType

B, CI, CO, H, W, E, KK, RR = 16, 128, 128, 128, 128, 16, 3, 8
NCORES = 8
BL = B // NCORES  # 2 samples per core
EPS = 1e-5
HW = H * W
IKK = CI * KK * KK  # 1152
BNS = 1.0 / math.sqrt(1.0 + EPS)
# unpadded x layout: [2 guard][row0 zeros][rows 1..128 = x][row129 zeros][2 guard]
GF = 2                      # front guard elems (keeps rows 4B-aligned)
XROW = lambda r: GF + r * W  # flat offset of padded row r (0..129)
# extra tail slack so strided edge-column APs (base + h*W, h<128) stay
# in-bounds; the slack region itself is never read
GL = GF + 131 * W

_CACHE = {}


def _build_module():
    nc = bacc.Bacc("TRN2", target_bir_lowering=False, debug=False)

    # ---- external inputs (host-prepped layouts) ----
    xb_d = nc.dram_tensor("x2b", [BL, CI, H, W], BF16, kind="ExternalInput").ap()
    ew_d = nc.dram_tensor("experts_w", [16, 128, IKK], BF16, kind="ExternalInput").ap()
    ident_d = nc.dram_tensor("identb", [128, 128], BF16, kind="ExternalInput").ap()
    mcdh_d = nc.dram_tensor("mcdh", [128, 14 * 128], BF16, kind="ExternalInput").ap()
    emask_d = nc.dram_tensor("emask", [128, 16], F32, kind="ExternalInput").ap()
    rw1t_d = nc.dram_tensor("rw1t", [CI, 16], F32, kind="ExternalInput").ap()
    rw2t_d = nc.dram_tensor("rw2t", [16, CI], F32, kind="ExternalInput").ap()
    rw3t_d = nc.dram_tensor("rw3t", [CI, 16], F32, kind="ExternalInput").ap()
    caw1t_d = nc.dram_tensor("caw1t", [CO, 16], F32, kind="ExternalInput").ap()
    caw2t_d = nc.dram_tensor("caw2t", [16, CO], F32, kind="ExternalInput").ap()
    g1_d = nc.dram_tensor("rbn1_g", [16], F32, kind="ExternalInput").ap()
    b1_d = nc.dram_tensor("rbn1_b", [16], F32, kind="ExternalInput").ap()
    g2_d = nc.dram_tensor("rbn2_g", [CI], F32, kind="ExternalInput").ap()
    b2_d = nc.dram_tensor("rbn2_b", [CI], F32, kind="ExternalInput").ap()
    rb3_d = nc.dram_tensor("rb3", [E], F32, kind="ExternalInput").ap()
    cag1_d = nc.dram_tensor("ca_bn1_g", [16], F32, kind="ExternalInput").ap()
    cab1_d = nc.dram_tensor("ca_bn1_b", [16], F32, kind="ExternalInput").ap()
    cag2_d = nc.dram_tensor("ca_bn2_g", [CO], F32, kind="ExternalInput").ap()
    cab2_d = nc.dram_tensor("ca_bn2_b", [CO], F32, kind="ExternalInput").ap()
    sag_d = nc.dram_tensor("sa_bn_g", [1], F32, kind="ExternalInput").ap()
    sab_d = nc.dram_tensor("sa_bn_b", [1], F32, kind="ExternalInput").ap()
    bmask_d = nc.dram_tensor("bmask", [128, 8], BF16, kind="ExternalInput").ap()

    out_d = nc.dram_tensor("out", [BL, CO, H, W], F32, kind="ExternalOutput").ap()

    # internal DRAM scratch (spatial-attention map bounce for broadcast)
    ssw_d = nc.dram_tensor("scr_sw", [BL, H, W], BF16).ap()

    with tile.TileContext(nc) as tc, ExitStack() as ctx:
        _kernel_body(
            ctx, tc,
            xb_d, ew_d, ident_d, mcdh_d, emask_d, rw1t_d, rw2t_d, rw3t_d,
            caw1t_d, caw2t_d, g1_d, b1_d, g2_d, b2_d, rb3_d, cag1_d, cab1_d,
            cag2_d, cab2_d, sag_d, sab_d, bmask_d, out_d, ssw_d,
        )
    nc.compile()
    return nc


def _kernel_body(ctx, tc,
                 xb_d, ew_d, ident_d, mcdh_d, emask_d, rw1t_d, rw2t_d, rw3t_d,
                 caw1t_d, caw2t_d, g1_d, b1_d, g2_d, b2_d, rb3_d, cag1_d,
                 cab1_d, cag2_d, cab2_d, sag_d, sab_d, bmask_d, out_d, ssw_d):
    nc = tc.nc

    cpool = ctx.enter_context(tc.tile_pool(name="const", bufs=1))
    xpool = ctx.enter_context(tc.tile_pool(name="xp", bufs=2))
    opool = ctx.enter_context(tc.tile_pool(name="op", bufs=2))
    wpool = ctx.enter_context(tc.tile_pool(name="wp", bufs=2))
    epool = ctx.enter_context(tc.tile_pool(name="ep", bufs=7))
    spool = ctx.enter_context(tc.tile_pool(name="sp", bufs=2))
    fpre = ctx.enter_context(tc.tile_pool(name="fpre", bufs=2))
    fpool = ctx.enter_context(tc.tile_pool(name="fp", bufs=3))
    fof = ctx.enter_context(tc.tile_pool(name="fof", bufs=3))

    pconv = ctx.enter_context(tc.tile_pool(name="pc", bufs=4, space="PSUM"))
    ptp = ctx.enter_context(tc.tile_pool(name="pt", bufs=2, space="PSUM"))
    pw = ctx.enter_context(tc.tile_pool(name="pw", bufs=1, space="PSUM"))
    pr = ctx.enter_context(tc.tile_pool(name="prt", bufs=1, space="PSUM"))

    # ---------- x load for sample 0 first: its DMAs + pool partial sums
    # must not queue behind the constant-loading storm ----------
    xus = [None, None]
    pparts_t = [None, None]

    def stage_load_pool(b):
        xu = xpool.tile([128, GL], BF16, tag="x_un", name=f"xu{b}")
        xus[b] = xu
        nc.vector.memset(xu[:, 0:XROW(1)], 0.0)
        nc.vector.memset(xu[:, XROW(129):GL], 0.0)
        pparts = spool.tile([128, 8], F32, tag="pparts", name=f"pparts{b}")
        pparts_t[b] = pparts
        for t in range(8):
            lo = XROW(1 + 16 * t)
            chunk = xu[:, lo:lo + 16 * W]
            nc.sync.dma_start(chunk, xb_d[b, :, 16 * t:16 * t + 16, :])
            if t % 2 == 0:
                nc.vector.tensor_reduce(pparts[:, t:t + 1], chunk, AX.X,
                                        ALU.add)
            else:
                # in-place ACT copy; accum_out collects the chunk sum
                nc.scalar.activation(chunk, chunk, ACTF.Copy,
                                     accum_out=pparts[:, t:t + 1])

    stage_load_pool(0)

    # ---------- constants ----------
    ident = cpool.tile([128, 128], BF16, tag="ident")
    nc.sync.dma_start(ident, ident_d)
    mcdh = cpool.tile([128, 14 * 128], BF16, tag="mcdh")
    nc.sync.dma_start(mcdh, mcdh_d)
    emask = cpool.tile([128, 16], F32, tag="emask")
    nc.sync.dma_start(emask, emask_d)
    ones1 = cpool.tile([1, 128], F32, tag="ones1")
    nc.vector.memset(ones1, 1.0)

    rw1t = cpool.tile([CI, 16], F32, tag="rw1t")
    nc.sync.dma_start(rw1t, rw1t_d)
    rw2t = cpool.tile([16, CI], F32, tag="rw2t")
    nc.sync.dma_start(rw2t, rw2t_d)
    rw3t = cpool.tile([CI, 16], F32, tag="rw3t")
    nc.sync.dma_start(rw3t, rw3t_d)
    caw1t = cpool.tile([CO, 16], F32, tag="caw1t")
    nc.sync.dma_start(caw1t, caw1t_d)
    caw2t = cpool.tile([16, CO], F32, tag="caw2t")
    nc.sync.dma_start(caw2t, caw2t_d)

    def vec_const(dst_tag, src_ap, n, scale):
        raw = cpool.tile([n, 1], F32, tag=dst_tag + "_r")
        nc.sync.dma_start(raw, src_ap.unsqueeze(1))
        out = cpool.tile([n, 1], F32, tag=dst_tag)
        nc.vector.tensor_scalar_mul(out, raw, float(scale))
        return out

    gs1 = vec_const("gs1", g1_d, 16, BNS / HW)
    bb1 = vec_const("bb1", b1_d, 16, 1.0)
    gs2 = vec_const("gs2", g2_d, CI, BNS)
    bb2 = vec_const("bb2", b2_d, CI, 1.0)
    gsca1 = vec_const("gsca1", cag1_d, 16, BNS / HW)
    bbca1 = vec_const("bbca1", cab1_d, 16, 1.0)
    gsca2 = vec_const("gsca2", cag2_d, CO, BNS)
    bbca2 = vec_const("bbca2", cab2_d, CO, 1.0)

    rb3r = cpool.tile([1, E], F32, tag="rb3r")
    nc.sync.dma_start(rb3r, rb3_d.unsqueeze(0))

    gssa = cpool.tile([128, 1], F32, tag="gssa")
    nc.sync.dma_start(gssa, sag_d.unsqueeze(0).partition_broadcast(128))
    nc.vector.tensor_scalar_mul(gssa, gssa, BNS)
    bssa = cpool.tile([128, 1], F32, tag="bssa")
    nc.sync.dma_start(bssa, sab_d.unsqueeze(0).partition_broadcast(128))
    bmask = cpool.tile([128, 8], BF16, tag="bmask")
    nc.sync.dma_start(bmask, bmask_d)

    # ---------- per-sample state ----------
    osbs = [None, None]
    wsbs = [None, None]
    cparts = [None, None]
    stats = [None, None]

    def stage_route(b):
        """routing MLP -> softmax -> block-diag rwblk."""
        pparts = pparts_t[b]
        psum_t = spool.tile([128, 1], F32, tag="psum_t")
        nc.scalar.activation(pparts, pparts, ACTF.Copy, accum_out=psum_t)

        # -- routing MLP (f32) --
        mm1 = pr.tile([16, 1], F32, tag="r")
        nc.tensor.matmul(mm1, rw1t, psum_t, start=True, stop=True)
        h1 = spool.tile([16, 1], F32, tag="h1")
        nc.scalar.activation(h1, mm1, ACTF.Relu, bias=bb1, scale=gs1)
        mm2 = pr.tile([128, 1], F32, tag="r")
        nc.tensor.matmul(mm2, rw2t, h1, start=True, stop=True)
        gg = spool.tile([128, 1], F32, tag="gg")
        nc.scalar.activation(gg, mm2, ACTF.Sigmoid, bias=bb2, scale=gs2)
        mm3 = pr.tile([1, E], F32, tag="r")
        nc.tensor.matmul(mm3, gg, rw3t, start=True, stop=True)
        lg = spool.tile([1, E], F32, tag="lg")
        nc.vector.tensor_add(lg, mm3, rb3r)
        mx = spool.tile([1, 1], F32, tag="mx")
        nc.vector.tensor_reduce(mx, lg, AX.X, ALU.max)
        mxn = spool.tile([1, 1], F32, tag="mxn")
        nc.vector.tensor_scalar_mul(mxn, mx, -1.0)
        e16 = spool.tile([1, E], F32, tag="e16")
        nc.scalar.activation(e16, lg, ACTF.Exp, bias=mxn, scale=1.0)
        s1 = spool.tile([1, 1], F32, tag="s1")
        nc.vector.tensor_reduce(s1, e16, AX.X, ALU.add)
        rinv = spool.tile([1, 1], F32, tag="rinv")
        nc.vector.reciprocal(rinv, s1)
        rwrow = spool.tile([1, E], F32, tag="rwrow")
        nc.vector.tensor_scalar_mul(rwrow, e16, rinv)

        # on-chip broadcast: rank-1 PE outer product + masked select
        rwbp = pr.tile([128, 16], F32, tag="r")
        nc.tensor.matmul(rwbp, ones1, rwrow, start=True, stop=True)
        rwsel = spool.tile([128, 16], F32, tag="rwsel")
        nc.vector.tensor_mul(rwsel, rwbp, emask)
        rwcol = spool.tile([128, 1], F32, tag="rwcol")
        nc.vector.tensor_reduce(rwcol, rwsel, AX.X, ALU.add)
        rwblk = spool.tile([128, 8], BF16, tag="rwblk")
        nc.vector.tensor_scalar_mul(rwblk, bmask, rwcol)
        return rwblk

    def stage_wgen(b, rwblk):
        """w[i, k, o] = sum_e rw[e] experts[e, o, i, k] via block-diag MMs."""
        wsb = wpool.tile([128, KK * KK, CO], BF16, tag="wsb")
        wsbs[b] = wsb
        for og in range(16):
            ec = epool.tile([128, IKK], BF16, tag="echunk")
            nc.sync.dma_start(ec, ew_d[og])
            eck = ec.rearrange("p (k i) -> p k i", k=9)
            pwt = pw.tile([128, 9, 8], F32, tag="w")
            for k in range(9):
                nc.tensor.matmul(pwt[:, k, :], eck[:, k, :], rwblk,
                                 start=True, stop=True)
            nc.scalar.copy(wsb[:, :, og * 8:og * 8 + 8], pwt)

    def stage_conv_alloc(b):
        osbs[b] = opool.tile([128, H, W], BF16, tag="out_sb",
                             name=f"osb{b}")
        cparts[b] = spool.tile([128, 34], F32, tag="cparts",
                               name=f"cparts{b}")

    def emit_conv_edges(b):
        """exact conv output columns 0 and 127 (the bulk matmuls read
        wrapped garbage there); evicted into osb + edge channel sums."""
        xu, wsb, osb = xus[b], wsbs[b], osbs[b]
        cp = cparts[b]
        for side, kws, wcol in ((0, (1, 2), 0), (1, (0, 1), W - 1)):
            pe = ptp.tile([128, 128], F32, tag="t", name=f"pe{b}_{side}")
            first = True
            for kh in range(3):
                for kw in kws:
                    xcol = wcol + kw - 1
                    base = XROW(kh) + xcol
                    rhs = xu[:, base:base + HW].rearrange(
                        "p (a c) -> p a c", a=128)[:, :, 0]
                    nc.tensor.matmul(pe, wsb[:, kh * 3 + kw, :], rhs,
                                     start=first, stop=(kh == 2 and kw == kws[-1]))
                    first = False
            nc.scalar.activation(osb[:, :, wcol], pe, ACTF.Copy,
                                 accum_out=cp[:, 32 + side:33 + side])

    def emit_conv_super(b, sup):
        """one 16-row super: 4 PSUM groups x 9 taps, evict interior."""
        xu, wsb, osb, cp = xus[b], wsbs[b], osbs[b], cparts[b]
        pcs = [pconv.tile([128, 512], F32, tag="c", name=f"pc{b}_{sup}_{i}")
               for i in range(4)]
        for k in range(9):
            kh, kw = k // 3, k % 3
            lhs = wsb[:, k, :]
            for g in range(4):
                base = XROW(sup * 16 + g * 4 + kh) + kw - 1
                rhs = xu[:, base:base + 4 * W].rearrange(
                    "p (a c) -> p a c", a=4)
                nc.tensor.matmul(pcs[g], lhs, rhs,
                                 start=(k == 0), stop=(k == 8))
        for g in range(4):
            hr = sup * 16 + g * 4
            src = pcs[g].rearrange("p (a c) -> p a c", a=4)[:, :, 1:W - 1]
            nc.scalar.activation(
                osb[:, hr:hr + 4, 1:W - 1], src,
                ACTF.Copy, accum_out=cp[:, sup * 4 + g:sup * 4 + g + 1])

    def stage_se(b, split_scale=False):
        """SE MLP on accumulated channel sums -> cw, in-place bf16 scale."""
        osb, cp = osbs[b], cparts[b]
        cps = spool.tile([128, 1], F32, tag="cps")
        nc.scalar.activation(cp, cp, ACTF.Copy, accum_out=cps)
        se1 = pr.tile([16, 1], F32, tag="r")
        nc.tensor.matmul(se1, caw1t, cps, start=True, stop=True)
        ch = spool.tile([16, 1], F32, tag="ch")
        nc.scalar.activation(ch, se1, ACTF.Relu, bias=bbca1, scale=gsca1)
        se2 = pr.tile([128, 1], F32, tag="r")
        nc.tensor.matmul(se2, caw2t, ch, start=True, stop=True)
        cw = spool.tile([128, 1], F32, tag="cw")
        nc.scalar.activation(cw, se2, ACTF.Sigmoid, bias=bbca2, scale=gsca2)
        for g in range(8):
            sl = osb[:, 16 * g:16 * g + 16, :]
            if split_scale and g < 2:
                nc.scalar.mul(sl, sl, cw)
            else:
                nc.vector.tensor_scalar_mul(sl, sl, cw)

    def emit_stat_init(b):
        spmax = spool.tile([128, 134], BF16, tag="spmax", name=f"spmax{b}")
        spsum = spool.tile([128, 134], BF16, tag="spsum", name=f"spsum{b}")
        stats[b] = (spmax, spsum)
        nc.vector.memset(spmax[:, 0:3], 0.0)
        nc.vector.memset(spmax[:, 131:134], 0.0)
        nc.vector.memset(spsum[:, 0:3], 0.0)
        nc.vector.memset(spsum[:, 131:134], 0.0)

    def emit_stat_chunk(b, q):
        """transpose 8 h-rows, reduce channel max / sum into [w, h] maps."""
        osb = osbs[b]
        spmax, spsum = stats[b]
        ptt = ptp.tile([128, 1024], BF16, tag="t")
        for j in range(8):
            nc.tensor.transpose(
                ptt[:, 128 * j:128 * (j + 1)], osb[:, 8 * q + j, :], ident)
        v = ptt.rearrange("p (a c) -> p a c", a=8)
        nc.vector.tensor_reduce(spmax[:, 3 + 8 * q:11 + 8 * q], v, AX.X, ALU.max)
        with nc.allow_low_precision(reason="bf16 channel-sum for 7x7 attn"):
            nc.vector.tensor_reduce(spsum[:, 3 + 8 * q:11 + 8 * q], v, AX.X,
                                    ALU.add)

    def emit_banded(b, half=None):
        """7x7 spatial conv as 14 banded-Toeplitz matmuls + sigmoid.
        half=0/1 computes only output rows [0,64) / [64,128) so the
        sw-map DRAM bounce can start before all stat chunks finish."""
        spmax, spsum = stats[b]
        lo, n = (0, 128) if half is None else (64 * half, 64)
        psw = ptp.tile([128, 128], F32, tag="t", name=f"psw{b}_{half}")
        for t in range(14):
            c, dh = t // 7, t % 7
            src = spsum if c == 0 else spmax
            nc.tensor.matmul(psw[:, 0:n], mcdh[:, t * 128:(t + 1) * 128],
                             src[:, lo + dh:lo + dh + n],
                             start=(t == 0), stop=(t == 13))
        swT = spool.tile([128, 128], BF16, tag="swT", name=f"swT{b}_{half}")
        nc.scalar.activation(swT[:, 0:n], psw[:, 0:n], ACTF.Sigmoid,
                             bias=bssa, scale=gssa)
        pswh = ptp.tile([128, 128], BF16, tag="t", name=f"pswh{b}_{half}")
        nc.tensor.transpose(pswh[0:n, :], swT[:, 0:n], ident)
        swH = spool.tile([128, 128], BF16, tag="swH", name=f"swH{b}_{half}")
        nc.vector.tensor_copy(swH[0:n, :], pswh[0:n, :])
        nc.sync.dma_start(ssw_d[b, lo:lo + n, :], swH[0:n, :])

    def emit_final_chunk(b, t):
        """out[:, 16t:16t+16, :] = osb*sw + x (bf16), cast f32, store."""
        xu, osb = xus[b], osbs[b]
        r0 = 16 * t
        eng = nc.vector
        swbc = fpre.tile([128, 16, W], BF16, tag="swbc")
        nc.sync.dma_start(
            swbc, ssw_d[b, r0:r0 + 16, :].partition_broadcast(128))
        tmul = fpool.tile([128, 16, W], BF16, tag="tmul")
        eng.tensor_tensor(tmul, osb[:, r0:r0 + 16, :], swbc, ALU.mult)
        lo = XROW(1 + r0)
        xres = xu[:, lo:lo + 16 * W].rearrange("p (a c) -> p a c", a=16)
        eng.tensor_tensor(tmul, tmul, xres, ALU.add)
        fo = fof.tile([128, 16, W], F32, tag="fo")
        nc.scalar.copy(fo, tmul)
        nc.sync.dma_start(out_d[b, :, r0:r0 + 16, :], fo)

    # ---------- pipelined schedule over the two samples ----------
    rwblk0 = stage_route(0)
    stage_wgen(0, rwblk0)
    stage_conv_alloc(0)
    emit_conv_edges(0)
    for sup in range(8):
        emit_conv_super(0, sup)
    stage_load_pool(1)
    rwblk1 = stage_route(1)
    stage_wgen(1, rwblk1)
    stage_conv_alloc(1)
    emit_conv_edges(1)
    stage_se(0)
    emit_stat_init(0)

    # sample 1 conv interleaved with sample 0 CBAM + final
    for sup in range(8):
        emit_conv_super(1, sup)
        if sup < 4:
            for qq in range(4):
                emit_stat_chunk(0, 4 * sup + qq)
        elif sup == 4:
            emit_banded(0)
        elif sup == 5:
            for t in range(4):
                emit_final_chunk(0, t)
        elif sup == 6:
            for t in range(4, 8):
                emit_final_chunk(0, t)

    # sample 1 post-conv tail
    stage_se(1, split_scale=True)
    emit_stat_init(1)
    for q in range(9):
        emit_stat_chunk(1, q)
    emit_banded(1, half=0)
    for q in range(9, 16):
        emit_stat_chunk(1, q)
    emit_banded(1, half=1)
    for t in range(8):
        emit_final_chunk(1, t)


def _host_prep(inp):
    import ml_dtypes
    experts = np.ascontiguousarray(inp["experts"], dtype=np.float32)
    # ew2[og][j'*16+e][k*128+i] = experts[e, og*8+j', i, kh, kw], k=kh*3+kw
    ew = experts.reshape(E, 16, 8, CI, 9)
    ew = np.ascontiguousarray(ew.transpose(1, 2, 0, 4, 3)).reshape(16, 128, IKK)

    identb = np.eye(128, dtype=np.float32)

    # banded-Toeplitz 7x7 attention matrices:
    # mcdh[c*7+dh][k, w] = sak[c, dh, k-w+3] for k-w+3 in [0,7), else 0
    saw = np.asarray(inp["sa_w"], np.float32).reshape(2, 7, 7)
    sak = saw.copy()
    sak[0] *= 1.0 / CO  # fold channel-mean normalization into mean taps
    mc = np.zeros((14, 128, 128), dtype=np.float32)
    kk, ww = np.meshgrid(np.arange(128), np.arange(128), indexing="ij")
    dwi = kk - ww + 3
    band = (dwi >= 0) & (dwi < 7)
    for c in range(2):
        for dh in range(7):
            m = np.zeros((128, 128), dtype=np.float32)
            m[band] = sak[c, dh, dwi[band]]
            mc[c * 7 + dh] = m
    mcdh = np.ascontiguousarray(mc.transpose(1, 0, 2)).reshape(128, 14 * 128)

    bm = np.zeros((8, 16, 8), dtype=np.float32)
    for j in range(8):
        bm[j, :, j] = 1.0
    bm = bm.reshape(128, 8)

    # emask[p, e] = 1 if e == p % 16 (block-diag expert select)
    em = np.zeros((128, 16), dtype=np.float32)
    em[np.arange(128), np.arange(128) % 16] = 1.0

    shared = {
        "experts_w": ew.astype(ml_dtypes.bfloat16),
        "identb": identb.astype(ml_dtypes.bfloat16),
        "mcdh": mcdh.astype(ml_dtypes.bfloat16),
        "emask": em,
        "rw1t": np.ascontiguousarray(inp["rw1"].T, dtype=np.float32),
        "rw2t": np.ascontiguousarray(inp["rw2"].T, dtype=np.float32),
        "rw3t": np.ascontiguousarray(inp["rw3"].T, dtype=np.float32),
        "caw1t": np.ascontiguousarray(inp["ca_w1"].T, dtype=np.float32),
        "caw2t": np.ascontiguousarray(inp["ca_w2"].T, dtype=np.float32),
        "rbn1_g": np.asarray(inp["rbn1_g"], np.float32),
        "rbn1_b": np.asarray(inp["rbn1_b"], np.float32),
        "rbn2_g": np.asarray(inp["rbn2_g"], np.float32),
        "rbn2_b": np.asarray(inp["rbn2_b"], np.float32),
        "rb3": np.asarray(inp["rb3"], np.float32),
        "ca_bn1_g": np.asarray(inp["ca_bn1_g"], np.float32),
        "ca_bn1_b": np.asarray(inp["ca_bn1_b"], np.float32),
        "ca_bn2_g": np.asarray(inp["ca_bn2_g"], np.float32),
        "ca_bn2_b": np.asarray(inp["ca_bn2_b"], np.float32),
        "sa_bn_g": np.asarray(inp["sa_bn_g"], np.float32),
        "sa_bn_b": np.asarray(inp["sa_bn_b"], np.float32),
        "bmask": bm.astype(ml_dtypes.bfloat16),
    }
    x = np.asarray(inp["x"], np.float32)
    in_maps = []
    for c in range(NCORES):
        m = dict(shared)
        xc = np.ascontiguousarray(x[BL * c:BL * (c + 1)])
        m["x2b"] = xc.astype(ml_dtypes.bfloat16)
        in_maps.append(m)
    return in_maps


def get_module():
    if "nc" not in _CACHE:
        _CACHE["nc"] = _build_module()
    return _CACHE["nc"]


def kernel(**inputs):
    nc = get_module()
    in_maps = _host_prep(inputs)
    res = run_bass_kernel_spmd(nc, in_maps, core_ids=list(range(NCORES)))
    out = np.concatenate([r["out"] for r in res.results], axis=0)
    return out.astype(np.float32)


# revision 22
# speedup vs baseline: 1.0265x; 1.0237x over previous
"""Trainium2 Bass kernel for EnhancedCondConv2d (moe_routing).

Data-parallel over batch: 8 cores x 2 samples each. Full inputs in,
full outputs back.

Per-core program (per sample):
  1. routing: avgpool(x) -> tiny MLP -> softmax -> rweights [16]
  2. w[b] = sum_e rweights[e] * experts[e]  (block-diag PE matmuls)
  3. 3x3 grouped conv as 9 PSUM-accumulated shifted matmuls (bf16).
     x is stored UNPADDED (contiguous rows, line-rate DMA) with only
     guard columns; the W-edge wrap contaminates output columns 0 and
     127, which are recomputed exactly via 12 small edge matmuls and
     evicted separately (so SE channel sums stay exact).
  4. SE: channel mean folded into PSUM eviction (ACT accum), MLP -> cw,
     in-place bf16 DVE scale pass osb *= cw
  5. CBAM: PE transposes (bf16) -> DVE max / sum over channels -> 7x7
     conv as 14 banded-Toeplitz matmuls (host-precomputed bands)
  6. final: out = osb*sw + x in bf16 (contiguous reads), ACT cast to
     f32, DMA out

Pipelining: sample 0's CBAM/final chunks are emitted interleaved with
sample 1's conv supers so sample 0's DVE-heavy post-conv phase hides
under sample 1's PE-heavy conv.
"""

import math
from contextlib import ExitStack

import numpy as np

import concourse.bass as bass
import concourse.bacc as bacc
import concourse.mybir as mybir
import concourse.tile as tile
from concourse.bass_utils import run_bass_kernel_spmd

F32 = mybir.dt.float32
BF16 = mybir.dt.bfloat16
AX = mybir.AxisListType
ALU = mybir.AluOpType
ACTF = mybir.ActivationFunctionType

B, CI, CO, H, W, E, KK, RR = 16, 128, 128, 128, 128, 16, 3, 8
NCORES = 8
BL = B // NCORES  # 2 samples per core
EPS = 1e-5
HW = H * W
IKK = CI * KK * KK  # 1152
BNS = 1.0 / math.sqrt(1.0 + EPS)
# unpadded x layout: [2 guard][row0 zeros][rows 1..128 = x][row129 zeros][2 guard]
GF = 2                      # front guard elems (keeps rows 4B-aligned)
XROW = lambda r: GF + r * W  # flat offset of padded row r (0..129)
# extra tail slack so strided edge-column APs (base + h*W, h<128) stay
# in-bounds; the slack region itself is never read
GL = GF + 131 * W

_CACHE = {}


def _build_module():
    nc = bacc.Bacc("TRN2", target_bir_lowering=False, debug=False)

    # ---- external inputs (host-prepped layouts) ----
    xb_d = nc.dram_tensor("x2b", [BL, CI, H, W], BF16, kind="ExternalInput").ap()
    ew_d = nc.dram_tensor("experts_w", [16, 128, IKK], BF16, kind="ExternalInput").ap()
    ident_d = nc.dram_tensor("identb", [128, 128], BF16, kind="ExternalInput").ap()
    mcdh_d = nc.dram_tensor("mcdh", [128, 14 * 128], BF16, kind="ExternalInput").ap()
    emask_d = nc.dram_tensor("emask", [128, 16], F32, kind="ExternalInput").ap()
    rw1t_d = nc.dram_tensor("rw1t", [CI, 16], F32, kind="ExternalInput").ap()
    rw2t_d = nc.dram_tensor("rw2t", [16, CI], F32, kind="ExternalInput").ap()
    rw3t_d = nc.dram_tensor("rw3t", [CI, 16], F32, kind="ExternalInput").ap()
    caw1t_d = nc.dram_tensor("caw1t", [CO, 16], F32, kind="ExternalInput").ap()
    caw2t_d = nc.dram_tensor("caw2t", [16, CO], F32, kind="ExternalInput").ap()
    g1_d = nc.dram_tensor("rbn1_g", [16], F32, kind="ExternalInput").ap()
    b1_d = nc.dram_tensor("rbn1_b", [16], F32, kind="ExternalInput").ap()
    g2_d = nc.dram_tensor("rbn2_g", [CI], F32, kind="ExternalInput").ap()
    b2_d = nc.dram_tensor("rbn2_b", [CI], F32, kind="ExternalInput").ap()
    rb3_d = nc.dram_tensor("rb3", [E], F32, kind="ExternalInput").ap()
    cag1_d = nc.dram_tensor("ca_bn1_g", [16], F32, kind="ExternalInput").ap()
    cab1_d = nc.dram_tensor("ca_bn1_b", [16], F32, kind="ExternalInput").ap()
    cag2_d = nc.dram_tensor("ca_bn2_g", [CO], F32, kind="ExternalInput").ap()
    cab2_d = nc.dram_tensor("ca_bn2_b", [CO], F32, kind="ExternalInput").ap()
    sag_d = nc.dram_tensor("sa_bn_g", [1], F32, kind="ExternalInput").ap()
    sab_d = nc.dram_tensor("sa_bn_b", [1], F32, kind="ExternalInput").ap()
    bmask_d = nc.dram_tensor("bmask", [128, 8], BF16, kind="ExternalInput").ap()

    out_d = nc.dram_tensor("out", [BL, CO, H, W], F32, kind="ExternalOutput").ap()

    # internal DRAM scratch (spatial-attention map bounce for broadcast)
    ssw_d = nc.dram_tensor("scr_sw", [BL, H, W], BF16).ap()

    with tile.TileContext(nc) as tc, ExitStack() as ctx:
        _kernel_body(
            ctx, tc,
            xb_d, ew_d, ident_d, mcdh_d, emask_d, rw1t_d, rw2t_d, rw3t_d,
            caw1t_d, caw2t_d, g1_d, b1_d, g2_d, b2_d, rb3_d, cag1_d, cab1_d,
            cag2_d, cab2_d, sag_d, sab_d, bmask_d, out_d, ssw_d,
        )
    nc.compile()
    return nc


def _kernel_body(ctx, tc,
                 xb_d, ew_d, ident_d, mcdh_d, emask_d, rw1t_d, rw2t_d, rw3t_d,
                 caw1t_d, caw2t_d, g1_d, b1_d, g2_d, b2_d, rb3_d, cag1_d,
                 cab1_d, cag2_d, cab2_d, sag_d, sab_d, bmask_d, out_d, ssw_d):
    nc = tc.nc

    cpool = ctx.enter_context(tc.tile_pool(name="const", bufs=1))
    xpool = ctx.enter_context(tc.tile_pool(name="xp", bufs=2))
    opool = ctx.enter_context(tc.tile_pool(name="op", bufs=2))
    wpool = ctx.enter_context(tc.tile_pool(name="wp", bufs=2))
    epool = ctx.enter_context(tc.tile_pool(name="ep", bufs=7))
    spool = ctx.enter_context(tc.tile_pool(name="sp", bufs=2))
    fpre = ctx.enter_context(tc.tile_pool(name="fpre", bufs=2))
    fpool = ctx.enter_context(tc.tile_pool(name="fp", bufs=3))
    fof = ctx.enter_context(tc.tile_pool(name="fof", bufs=3))

    pconv = ctx.enter_context(tc.tile_pool(name="pc", bufs=4, space="PSUM"))
    ptp = ctx.enter_context(tc.tile_pool(name="pt", bufs=2, space="PSUM"))
    pw = ctx.enter_context(tc.tile_pool(name="pw", bufs=1, space="PSUM"))
    pr = ctx.enter_context(tc.tile_pool(name="prt", bufs=1, space="PSUM"))

    # ---------- x load for sample 0 first: its DMAs + pool partial sums
    # must not queue behind the constant-loading storm ----------
    xus = [None, None]
    pparts_t = [None, None]

    def stage_load_pool(b):
        xu = xpool.tile([128, GL], BF16, tag="x_un", name=f"xu{b}")
        xus[b] = xu
        nc.vector.memset(xu[:, 0:XROW(1)], 0.0)
        nc.vector.memset(xu[:, XROW(129):GL], 0.0)
        pparts = spool.tile([128, 8], F32, tag="pparts", name=f"pparts{b}")
        pparts_t[b] = pparts
        for t in range(8):
            lo = XROW(1 + 16 * t)
            chunk = xu[:, lo:lo + 16 * W]
            nc.sync.dma_start(chunk, xb_d[b, :, 16 * t:16 * t + 16, :])
            if t % 2 == 0:
                nc.vector.tensor_reduce(pparts[:, t:t + 1], chunk, AX.X,
                                        ALU.add)
            else:
                # in-place ACT copy; accum_out collects the chunk sum
                nc.scalar.activation(chunk, chunk, ACTF.Copy,
                                     accum_out=pparts[:, t:t + 1])

    stage_load_pool(0)

    # ---------- constants ----------
    ident = cpool.tile([128, 128], BF16, tag="ident")
    nc.sync.dma_start(ident, ident_d)
    mcdh = cpool.tile([128, 14 * 128], BF16, tag="mcdh")
    nc.sync.dma_start(mcdh, mcdh_d)
    emask = cpool.tile([128, 16], F32, tag="emask")
    nc.sync.dma_start(emask, emask_d)
    ones1 = cpool.tile([1, 128], F32, tag="ones1")
    nc.vector.memset(ones1, 1.0)

    rw1t = cpool.tile([CI, 16], F32, tag="rw1t")
    nc.sync.dma_start(rw1t, rw1t_d)
    rw2t = cpool.tile([16, CI], F32, tag="rw2t")
    nc.sync.dma_start(rw2t, rw2t_d)
    rw3t = cpool.tile([CI, 16], F32, tag="rw3t")
    nc.sync.dma_start(rw3t, rw3t_d)
    caw1t = cpool.tile([CO, 16], F32, tag="caw1t")
    nc.sync.dma_start(caw1t, caw1t_d)
    caw2t = cpool.tile([16, CO], F32, tag="caw2t")
    nc.sync.dma_start(caw2t, caw2t_d)

    def vec_const(dst_tag, src_ap, n, scale):
        raw = cpool.tile([n, 1], F32, tag=dst_tag + "_r")
        nc.sync.dma_start(raw, src_ap.unsqueeze(1))
        out = cpool.tile([n, 1], F32, tag=dst_tag)
        nc.vector.tensor_scalar_mul(out, raw, float(scale))
        return out

    gs1 = vec_const("gs1", g1_d, 16, BNS / HW)
    bb1 = vec_const("bb1", b1_d, 16, 1.0)
    gs2 = vec_const("gs2", g2_d, CI, BNS)
    bb2 = vec_const("bb2", b2_d, CI, 1.0)
    gsca1 = vec_const("gsca1", cag1_d, 16, BNS / HW)
    bbca1 = vec_const("bbca1", cab1_d, 16, 1.0)
    gsca2 = vec_const("gsca2", cag2_d, CO, BNS)
    bbca2 = vec_const("bbca2", cab2_d, CO, 1.0)

    rb3r = cpool.tile([1, E], F32, tag="rb3r")
    nc.sync.dma_start(rb3r, rb3_d.unsqueeze(0))

    gssa = cpool.tile([128, 1], F32, tag="gssa")
    nc.sync.dma_start(gssa, sag_d.unsqueeze(0).partition_broadcast(128))
    nc.vector.tensor_scalar_mul(gssa, gssa, BNS)
    bssa = cpool.tile([128, 1], F32, tag="bssa")
    nc.sync.dma_start(bssa, sab_d.unsqueeze(0).partition_broadcast(128))
    bmask = cpool.tile([128, 8], BF16, tag="bmask")
    nc.sync.dma_start(bmask, bmask_d)

    # ---------- per-sample state ----------
    osbs = [None, None]
    wsbs = [None, None]
    cparts = [None, None]
    stats = [None, None]

    def stage_route(b):
        """routing MLP -> softmax -> block-diag rwblk."""
        pparts = pparts_t[b]
        psum_t = spool.tile([128, 1], F32, tag="psum_t")
        nc.scalar.activation(pparts, pparts, ACTF.Copy, accum_out=psum_t)

        # -- routing MLP (f32) --
        mm1 = pr.tile([16, 1], F32, tag="r")
        nc.tensor.matmul(mm1, rw1t, psum_t, start=True, stop=True)
        h1 = spool.tile([16, 1], F32, tag="h1")
        nc.scalar.activation(h1, mm1, ACTF.Relu, bias=bb1, scale=gs1)
        mm2 = pr.tile([128, 1], F32, tag="r")
        nc.tensor.matmul(mm2, rw2t, h1, start=True, stop=True)
        gg = spool.tile([128, 1], F32, tag="gg")
        nc.scalar.activation(gg, mm2, ACTF.Sigmoid, bias=bb2, scale=gs2)
        mm3 = pr.tile([1, E], F32, tag="r")
        nc.tensor.matmul(mm3, gg, rw3t, start=True, stop=True)
        lg = spool.tile([1, E], F32, tag="lg")
        nc.vector.tensor_add(lg, mm3, rb3r)
        mx = spool.tile([1, 1], F32, tag="mx")
        nc.vector.tensor_reduce(mx, lg, AX.X, ALU.max)
        mxn = spool.tile([1, 1], F32, tag="mxn")
        nc.vector.tensor_scalar_mul(mxn, mx, -1.0)
        e16 = spool.tile([1, E], F32, tag="e16")
        nc.scalar.activation(e16, lg, ACTF.Exp, bias=mxn, scale=1.0)
        s1 = spool.tile([1, 1], F32, tag="s1")
        nc.vector.tensor_reduce(s1, e16, AX.X, ALU.add)
        rinv = spool.tile([1, 1], F32, tag="rinv")
        nc.vector.reciprocal(rinv, s1)
        rwrow = spool.tile([1, E], F32, tag="rwrow")
        nc.vector.tensor_scalar_mul(rwrow, e16, rinv)

        # on-chip broadcast: rank-1 PE outer product + masked select
        rwbp = pr.tile([128, 16], F32, tag="r")
        nc.tensor.matmul(rwbp, ones1, rwrow, start=True, stop=True)
        rwsel = spool.tile([128, 16], F32, tag="rwsel")
        nc.vector.tensor_mul(rwsel, rwbp, emask)
        rwcol = spool.tile([128, 1], F32, tag="rwcol")
        nc.vector.tensor_reduce(rwcol, rwsel, AX.X, ALU.add)
        rwblk = spool.tile([128, 8], BF16, tag="rwblk")
        nc.vector.tensor_scalar_mul(rwblk, bmask, rwcol)
        return rwblk

    def stage_wgen(b, rwblk):
        """w[i, k, o] = sum_e rw[e] experts[e, o, i, k] via block-diag MMs."""
        wsb = wpool.tile([128, KK * KK, CO], BF16, tag="wsb")
        wsbs[b] = wsb
        for og in range(16):
            ec = epool.tile([128, IKK], BF16, tag="echunk")
            nc.sync.dma_start(ec, ew_d[og])
            eck = ec.rearrange("p (k i) -> p k i", k=9)
            pwt = pw.tile([128, 9, 8], F32, tag="w")
            for k in range(9):
                nc.tensor.matmul(pwt[:, k, :], eck[:, k, :], rwblk,
                                 start=True, stop=True)
            nc.scalar.copy(wsb[:, :, og * 8:og * 8 + 8], pwt)

    def stage_conv_alloc(b):
        osbs[b] = opool.tile([128, H, W], BF16, tag="out_sb",
                             name=f"osb{b}")
        cparts[b] = spool.tile([128, 34], F32, tag="cparts",
                               name=f"cparts{b}")

    def emit_conv_edges(b):
        """exact conv output columns 0 and 127 (the bulk matmuls read
        wrapped garbage there); evicted into osb + edge channel sums."""
        xu, wsb, osb = xus[b], wsbs[b], osbs[b]
        cp = cparts[b]
        for side, kws, wcol in ((0, (1, 2), 0), (1, (0, 1), W - 1)):
            pe = ptp.tile([128, 128], F32, tag="t", name=f"pe{b}_{side}")
            first = True
            for kh in range(3):
                for kw in kws:
                    xcol = wcol + kw - 1
                    base = XROW(kh) + xcol
                    rhs = xu[:, base:base + HW].rearrange(
                        "p (a c) -> p a c", a=128)[:, :, 0]
                    nc.tensor.matmul(pe, wsb[:, kh * 3 + kw, :], rhs,
                                     start=first, stop=(kh == 2 and kw == kws[-1]))
                    first = False
            nc.scalar.activation(osb[:, :, wcol], pe, ACTF.Copy,
                                 accum_out=cp[:, 32 + side:33 + side])

    def emit_conv_super(b, sup):
        """one 16-row super: 4 PSUM groups x 9 taps, evict interior."""
        xu, wsb, osb, cp = xus[b], wsbs[b], osbs[b], cparts[b]
        pcs = [pconv.tile([128, 512], F32, tag="c", name=f"pc{b}_{sup}_{i}")
               for i in range(4)]
        for k in range(9):
            kh, kw = k // 3, k % 3
            lhs = wsb[:, k, :]
            for g in range(4):
                base = XROW(sup * 16 + g * 4 + kh) + kw - 1
                rhs = xu[:, base:base + 4 * W].rearrange(
                    "p (a c) -> p a c", a=4)
                nc.tensor.matmul(pcs[g], lhs, rhs,
                                 start=(k == 0), stop=(k == 8))
        for g in range(4):
            hr = sup * 16 + g * 4
            src = pcs[g].rearrange("p (a c) -> p a c", a=4)[:, :, 1:W - 1]
            nc.scalar.activation(
                osb[:, hr:hr + 4, 1:W - 1], src,
                ACTF.Copy, accum_out=cp[:, sup * 4 + g:sup * 4 + g + 1])

    def stage_se(b, split_scale=False):
        """SE MLP on accumulated channel sums -> cw, in-place bf16 scale."""
        osb, cp = osbs[b], cparts[b]
        cps = spool.tile([128, 1], F32, tag="cps")
        nc.scalar.activation(cp, cp, ACTF.Copy, accum_out=cps)
        se1 = pr.tile([16, 1], F32, tag="r")
        nc.tensor.matmul(se1, caw1t, cps, start=True, stop=True)
        ch = spool.tile([16, 1], F32, tag="ch")
        nc.scalar.activation(ch, se1, ACTF.Relu, bias=bbca1, scale=gsca1)
        se2 = pr.tile([128, 1], F32, tag="r")
        nc.tensor.matmul(se2, caw2t, ch, start=True, stop=True)
        cw = spool.tile([128, 1], F32, tag="cw")
        nc.scalar.activation(cw, se2, ACTF.Sigmoid, bias=bbca2, scale=gsca2)
        for g in range(8):
            sl = osb[:, 16 * g:16 * g + 16, :]
            if split_scale and g < 2:
                nc.scalar.mul(sl, sl, cw)
            else:
                nc.vector.tensor_scalar_mul(sl, sl, cw)

    def emit_stat_init(b):
        spmax = spool.tile([128, 134], BF16, tag="spmax", name=f"spmax{b}")
        spsum = spool.tile([128, 134], BF16, tag="spsum", name=f"spsum{b}")
        stats[b] = (spmax, spsum)
        nc.vector.memset(spmax[:, 0:3], 0.0)
        nc.vector.memset(spmax[:, 131:134], 0.0)
        nc.vector.memset(spsum[:, 0:3], 0.0)
        nc.vector.memset(spsum[:, 131:134], 0.0)

    def emit_stat_chunk(b, q):
        """transpose 8 h-rows, reduce channel max / sum into [w, h] maps."""
        osb = osbs[b]
        spmax, spsum = stats[b]
        ptt = ptp.tile([128, 1024], BF16, tag="t")
        for j in range(8):
            nc.tensor.transpose(
                ptt[:, 128 * j:128 * (j + 1)], osb[:, 8 * q + j, :], ident)
        v = ptt.rearrange("p (a c) -> p a c", a=8)
        nc.vector.tensor_reduce(spmax[:, 3 + 8 * q:11 + 8 * q], v, AX.X, ALU.max)
        with nc.allow_low_precision(reason="bf16 channel-sum for 7x7 attn"):
            nc.vector.tensor_reduce(spsum[:, 3 + 8 * q:11 + 8 * q], v, AX.X,
                                    ALU.add)

    def emit_banded(b, half=None):
        """7x7 spatial conv as 14 banded-Toeplitz matmuls + sigmoid.
        half=0/1 computes only output rows [0,64) / [64,128) so the
        sw-map DRAM bounce can start before all stat chunks finish."""
        spmax, spsum = stats[b]
        lo, n = (0, 128) if half is None else (64 * half, 64)
        psw = ptp.tile([128, 128], F32, tag="t", name=f"psw{b}_{half}")
        for t in range(14):
            c, dh = t // 7, t % 7
            src = spsum if c == 0 else spmax
            nc.tensor.matmul(psw[:, 0:n], mcdh[:, t * 128:(t + 1) * 128],
                             src[:, lo + dh:lo + dh + n],
                             start=(t == 0), stop=(t == 13))
        swT = spool.tile([128, 128], BF16, tag="swT", name=f"swT{b}_{half}")
        nc.scalar.activation(swT[:, 0:n], psw[:, 0:n], ACTF.Sigmoid,
                             bias=bssa, scale=gssa)
        pswh = ptp.tile([128, 128], BF16, tag="t", name=f"pswh{b}_{half}")
        nc.tensor.transpose(pswh[0:n, :], swT[:, 0:n], ident)
        swH = spool.tile([128, 128], BF16, tag="swH", name=f"swH{b}_{half}")
        nc.vector.tensor_copy(swH[0:n, :], pswh[0:n, :])
        nc.sync.dma_start(ssw_d[b, lo:lo + n, :], swH[0:n, :])

    def emit_final_chunk(b, t):
        """out[:, 16t:16t+16, :] = osb*sw + x (bf16), cast f32, store."""
        xu, osb = xus[b], osbs[b]
        r0 = 16 * t
        eng = nc.vector
        swbc = fpre.tile([128, 16, W], BF16, tag="swbc")
        nc.sync.dma_start(
            swbc, ssw_d[b, r0:r0 + 16, :].partition_broadcast(128))
        tmul = fpool.tile([128, 16, W], BF16, tag="tmul")
        eng.tensor_tensor(tmul, osb[:, r0:r0 + 16, :], swbc, ALU.mult)
        lo = XROW(1 + r0)
        xres = xu[:, lo:lo + 16 * W].rearrange("p (a c) -> p a c", a=16)
        eng.tensor_tensor(tmul, tmul, xres, ALU.add)
        fo = fof.tile([128, 16, W], F32, tag="fo")
        nc.scalar.copy(fo, tmul)
        nc.scalar.dma_start(out_d[b, :, r0:r0 + 16, :], fo)

    # ---------- pipelined schedule over the two samples ----------
    rwblk0 = stage_route(0)
    stage_wgen(0, rwblk0)
    stage_conv_alloc(0)
    emit_conv_edges(0)
    for sup in range(8):
        emit_conv_super(0, sup)
    stage_load_pool(1)
    rwblk1 = stage_route(1)
    stage_wgen(1, rwblk1)
    stage_conv_alloc(1)
    emit_conv_edges(1)
    stage_se(0)
    emit_stat_init(0)

    # sample 1 conv interleaved with sample 0 CBAM + final
    for sup in range(8):
        emit_conv_super(1, sup)
        if sup < 4:
            for qq in range(4):
                emit_stat_chunk(0, 4 * sup + qq)
        elif sup == 4:
            emit_banded(0)
        elif sup == 5:
            for t in range(4):
                emit_final_chunk(0, t)
        elif sup == 6:
            for t in range(4, 8):
                emit_final_chunk(0, t)

    # sample 1 post-conv tail
    stage_se(1, split_scale=True)
    emit_stat_init(1)
    for q in range(9):
        emit_stat_chunk(1, q)
    emit_banded(1, half=0)
    for q in range(9, 16):
        emit_stat_chunk(1, q)
    emit_banded(1, half=1)
    for t in range(8):
        emit_final_chunk(1, t)


def _host_prep(inp):
    import ml_dtypes
    experts = np.ascontiguousarray(inp["experts"], dtype=np.float32)
    # ew2[og][j'*16+e][k*128+i] = experts[e, og*8+j', i, kh, kw], k=kh*3+kw
    ew = experts.reshape(E, 16, 8, CI, 9)
    ew = np.ascontiguousarray(ew.transpose(1, 2, 0, 4, 3)).reshape(16, 128, IKK)

    identb = np.eye(128, dtype=np.float32)

    # banded-Toeplitz 7x7 attention matrices:
    # mcdh[c*7+dh][k, w] = sak[c, dh, k-w+3] for k-w+3 in [0,7), else 0
    saw = np.asarray(inp["sa_w"], np.float32).reshape(2, 7, 7)
    sak = saw.copy()
    sak[0] *= 1.0 / CO  # fold channel-mean normalization into mean taps
    mc = np.zeros((14, 128, 128), dtype=np.float32)
    kk, ww = np.meshgrid(np.arange(128), np.arange(128), indexing="ij")
    dwi = kk - ww + 3
    band = (dwi >= 0) & (dwi < 7)
    for c in range(2):
        for dh in range(7):
            m = np.zeros((128, 128), dtype=np.float32)
            m[band] = sak[c, dh, dwi[band]]
            mc[c * 7 + dh] = m
    mcdh = np.ascontiguousarray(mc.transpose(1, 0, 2)).reshape(128, 14 * 128)

    bm = np.zeros((8, 16, 8), dtype=np.float32)
    for j in range(8):
        bm[j, :, j] = 1.0
    bm = bm.reshape(128, 8)

    # emask[p, e] = 1 if e == p % 16 (block-diag expert select)
    em = np.zeros((128, 16), dtype=np.float32)
    em[np.arange(128), np.arange(128) % 16] = 1.0

    shared = {
        "experts_w": ew.astype(ml_dtypes.bfloat16),
        "identb": identb.astype(ml_dtypes.bfloat16),
        "mcdh": mcdh.astype(ml_dtypes.bfloat16),
        "emask": em,
        "rw1t": np.ascontiguousarray(inp["rw1"].T, dtype=np.float32),
        "rw2t": np.ascontiguousarray(inp["rw2"].T, dtype=np.float32),
        "rw3t": np.ascontiguousarray(inp["rw3"].T, dtype=np.float32),
        "caw1t": np.ascontiguousarray(inp["ca_w1"].T, dtype=np.float32),
        "caw2t": np.ascontiguousarray(inp["ca_w2"].T, dtype=np.float32),
        "rbn1_g": np.asarray(inp["rbn1_g"], np.float32),
        "rbn1_b": np.asarray(inp["rbn1_b"], np.float32),
        "rbn2_g": np.asarray(inp["rbn2_g"], np.float32),
        "rbn2_b": np.asarray(inp["rbn2_b"], np.float32),
        "rb3": np.asarray(inp["rb3"], np.float32),
        "ca_bn1_g": np.asarray(inp["ca_bn1_g"], np.float32),
        "ca_bn1_b": np.asarray(inp["ca_bn1_b"], np.float32),
        "ca_bn2_g": np.asarray(inp["ca_bn2_g"], np.float32),
        "ca_bn2_b": np.asarray(inp["ca_bn2_b"], np.float32),
        "sa_bn_g": np.asarray(inp["sa_bn_g"], np.float32),
        "sa_bn_b": np.asarray(inp["sa_bn_b"], np.float32),
        "bmask": bm.astype(ml_dtypes.bfloat16),
    }
    x = np.asarray(inp["x"], np.float32)
    in_maps = []
    for c in range(NCORES):
        m = dict(shared)
        xc = np.ascontiguousarray(x[BL * c:BL * (c + 1)])
        m["x2b"] = xc.astype(ml_dtypes.bfloat16)
        in_maps.append(m)
    return in_maps


def get_module():
    if "nc" not in _CACHE:
        _CACHE["nc"] = _build_module()
    return _CACHE["nc"]


def kernel(**inputs):
    nc = get_module()
    in_maps = _host_prep(inputs)
    res = run_bass_kernel_spmd(nc, in_maps, core_ids=list(range(NCORES)))
    out = np.concatenate([r["out"] for r in res.results], axis=0)
    return out.astype(np.float32)
